# revision 28
# baseline (speedup 1.0000x reference)
"""Trainium2 Bass kernel for nn_HODE_MDP (hypergraph ODE message passing).

Math (T_UP = T_GEO = T_P2P = 1.0, ALPHA = 0.8):
    pe  = poi_emb_weight[:-1]                      # [P, D]
    x/s/g = pe * sigmoid(pe @ W_t + b_t)           # col / seq / geo gates
    hg_pois    = x + HG_pu @ (HG_up @ x)
    geo_pois   = g + 0.4 * (poi_geo_graph @ g)
    trans_pois = s + HG_poi_src @ (HG_poi_tar @ s)
    hg_users   = (HG_up @ hg_pois)[user_idx]
    geo_users  = (HG_up @ geo_pois)[user_idx]
    out = concat([hg_pois, geo_pois, trans_pois, hg_users, geo_users])

Distribution (8 NeuronCores): row-shard every big matrix (P rows for
HG_pu / HG_poi_src / poi_geo_graph, U rows for HG_up, E rows for
HG_poi_tar).  Each core computes full gates in NATURAL k-tile layout
(stationary tiles for the PE) plus its own transposed block for the
output adds.  The intermediate [*, D] activations (y_up, y_tar) are
all-gathered in fp8; the final user aggregation is computed as
column-block partial sums (each core streams HG_up[:, own_rows] and
emits a [U, 2D] partial) that the host reduces — this removes the two
late AllGathers that serialized the baseline.

All big matrix streams are fp8e4 (e4m3) with power-of-2 pre-scaling
(matrices x 2^13, activations x 2^7) so values sit in the fp8 normal
range; matmuls run in DoubleRow perf mode (two 128-k-tiles per
instruction).  The ODE deltas are ~1e-2..1e-4 of the output scale, so
fp8 error on the deltas lands ~2e-4 end to end (gate: 2e-2).  PSUM is
f32; gates and output adds stay f32.  Host descales by the power-of-2
factors during assembly.
"""

import sys

if "/opt/trn_rl_repo" not in sys.path:
    sys.path.insert(0, "/opt/trn_rl_repo")

import numpy as np
import ml_dtypes

import concourse.bass as bass  # noqa: F401
import concourse.bacc as bacc
import concourse.mybir as mybir
import concourse.tile as tile
from concourse.bass_utils import run_bass_kernel_spmd

F32 = mybir.dt.float32
BF16 = mybir.dt.bfloat16
FP8 = mybir.dt.float8e4
SIG = mybir.ActivationFunctionType.Sigmoid
MULT = mybir.AluOpType.mult
ADD = mybir.AluOpType.add
BYPASS = mybir.AluOpType.bypass
DR = mybir.MatmulPerfMode.DoubleRow

NCORES = 8
P, U, E, D = 8192, 4096, 4096, 128
PP, UU, EE = P // NCORES, U // NCORES, E // NCORES  # 1024, 512, 512
KP, KU = P // 128, U // 128                         # 64, 32 k-tiles
RG = [list(range(NCORES))]

SX = 64.0         # activation scale 2^6 (y_up*2^19 must stay < fp8 max 240)
SM = 8192.0       # matrix scale 2^13
GEO_SCALE = 0.4   # ALPHA / 2 * T_GEO
# psum scales: y products 2^19; hg/trans 2^32; outputs carry 2^6
S_HG_OUT = 2.0 ** -26                 # 2^(6-32): psum -> 2^6-scaled delta
S_GEO_OUT = GEO_SCALE * 2.0 ** -13    # 2^(6-19)
S_USERS = 2.0 ** -19                  # host descale for user partials

_CACHE: dict = {}


def _build_nc():
    nc = bacc.Bacc(
        "TRN2",
        target_bir_lowering=False,
        debug=False,
        enable_asserts=False,
        num_devices=NCORES,
    )

    # ---- per-core DRAM inputs -------------------------------------------
    # peT: pe.T bf16 (z stationary); peN_s: natural k-tiles f32 scaled 2^7
    peT = nc.dram_tensor("peT", [D, P], BF16, kind="ExternalInput").ap()
    peN_s = nc.dram_tensor("peN_s", [128, KP, D], F32, kind="ExternalInput").ap()
    peTo_b = nc.dram_tensor("peTo_b", [D, PP], BF16, kind="ExternalInput").ap()
    peTo_s = nc.dram_tensor("peTo_s", [D, PP], F32, kind="ExternalInput").ap()
    wN = nc.dram_tensor("wN", [D, 3, D], BF16, kind="ExternalInput").ap()
    bB = nc.dram_tensor("bB", [128, 3, 4, D], BF16, kind="ExternalInput").ap()
    bT3 = nc.dram_tensor("bT3", [D, 3], F32, kind="ExternalInput").ap()
    ident = nc.dram_tensor("ident", [D, D], F32, kind="ExternalInput").ap()
    # fp8 streams in paired k-tile layout [128, n_k/2, 2, n_out]
    UpT = nc.dram_tensor("UpT", [128, KP // 2, 2, UU], FP8, kind="ExternalInput").ap()
    TarT = nc.dram_tensor("TarT", [128, KP // 2, 2, EE], FP8, kind="ExternalInput").ap()
    PuT = nc.dram_tensor("PuT", [128, KU // 2, 2, PP], FP8, kind="ExternalInput").ap()
    SrcT = nc.dram_tensor("SrcT", [128, KU // 2, 2, PP], FP8, kind="ExternalInput").ap()
    GeoT = nc.dram_tensor("GeoT", [128, KP // 2, 2, PP], FP8, kind="ExternalInput").ap()
    # users stream: u-chunk-major [128, 8 u-chunks, 4 pairs, 2, 512]
    UpC = nc.dram_tensor(
        "UpC", [128, U // 512, PP // 256, 2, 512], FP8, kind="ExternalInput"
    ).ap()

    # outputs: transposed pois (scaled 2^7) + user partials (scaled 2^20)
    poisT_o = nc.dram_tensor("poisT_o", [3, D, PP], F32, kind="ExternalOutput").ap()
    usersT_o = nc.dram_tensor(
        "usersT_o", [D, 2, U], BF16, kind="ExternalOutput"
    ).ap()

    with tile.TileContext(nc) as tc:
        with (
            tc.tile_pool(name="const", bufs=1) as constp,
            tc.tile_pool(name="big", bufs=2) as bigp,
            tc.tile_pool(name="rhs", bufs=2) as rhsp,
            tc.tile_pool(name="stage", bufs=3) as stagep,
            tc.tile_pool(name="outp", bufs=2) as outp,
            tc.tile_pool(name="psacc", bufs=2, space="PSUM") as psacc,
            tc.tile_pool(name="pz", bufs=2, space="PSUM") as pzp,
            tc.tile_pool(name="dram", bufs=1, space="DRAM") as dramp,
        ):
            # ---- collective bounce buffers (fp8 natural k-tiles) --------
            cc_yu_in = dramp.tile([128, UU // 128, D], FP8, name="cc_yu_in")
            cc_yu_out = dramp.tile(
                [NCORES * 128, UU // 128, D], FP8, addr_space="Shared",
                name="cc_yu_out",
            )
            cc_yt_in = dramp.tile([128, EE // 128, D], FP8, name="cc_yt_in")
            cc_yt_out = dramp.tile(
                [NCORES * 128, EE // 128, D], FP8, addr_space="Shared",
                name="cc_yt_out",
            )

            # ---- constants (small ones first; big ones split/off-queue) -
            sb_w = constp.tile([D, 3, D], BF16, name="sb_w")
            nc.scalar.dma_start(sb_w[:], wN)
            sb_bT = constp.tile([D, 3], F32, name="sb_bT")
            nc.scalar.dma_start(sb_bT[:], bT3)
            sb_ident = constp.tile([D, D], F32, name="sb_ident")
            nc.scalar.dma_start(sb_ident[:], ident)
            sb_peTo_b = constp.tile([D, PP], BF16, name="sb_peTo_b")
            nc.scalar.dma_start(sb_peTo_b[:], peTo_b)
            sb_peTo_s = constp.tile([D, PP], F32, name="sb_peTo_s")
            nc.scalar.dma_start(sb_peTo_s[:], peTo_s)
            sb_bB = constp.tile([128, 3, 4, D], BF16, name="sb_bB")
            nc.scalar.dma_start(sb_bB[:], bB)
            sb_peT = constp.tile([D, P], BF16, name="sb_peT")
            for h in range(4):
                cols = slice(P // 4 * h, P // 4 * (h + 1))
                nc.sync.dma_start(sb_peT[:, cols], peT[:, cols])
            sb_peN = constp.tile([128, KP, D], F32, name="sb_peN")
            for h in range(2):
                ks = slice(KP // 2 * h, KP // 2 * (h + 1))
                nc.gpsimd.dma_start(sb_peN[:, ks, :], peN_s[:, ks, :])

            # fp8 natural gate tiles (stationary operands), [128, KP, 128]
            sb_gate8 = [
                constp.tile([128, KP, D], FP8, name=f"sb_gate8_{t}")
                for t in range(3)
            ]
            # own-block transposed gates f32 (scaled 2^7) for the adds
            sb_gateT = [
                constp.tile([D, PP], F32, name=f"sb_gateT{t}") for t in range(3)
            ]

            def gate_full(t):
                """Full gate in natural k-tile layout, fp8 (scaled 2^7).

                z tiles [p,d] via stationary peT-tiles; bias added with a
                rank-1 (K=1) matmul into the same PSUM group; sigmoid and
                pe-mul in f32 (pe pre-scaled by 2^7 on host).
                """
                for c in range(KP // 4):
                    psz = pzp.tile([128, 4, D], F32, tag="pz")
                    for m in range(4):
                        k = 4 * c + m
                        nc.tensor.matmul(
                            psz[:, m, :],
                            sb_peT[:, k * 128 : (k + 1) * 128],
                            sb_w[:, t, :],
                            start=True, stop=True,
                        )
                    zb = stagep.tile([128, 4, D], F32, tag="zb")
                    nc.vector.tensor_tensor(zb[:], psz[:], sb_bB[:, t], ADD)
                    sig = stagep.tile([128, 4, D], F32, tag="sig")
                    nc.scalar.activation(sig[:], zb[:], SIG)
                    nc.gpsimd.tensor_mul(
                        sb_gate8[t][:, 4 * c : 4 * c + 4, :],
                        sb_peN[:, 4 * c : 4 * c + 4, :], sig[:],
                    )

            def gate_own(t):
                """Own-block transposed gate (f32, scaled 2^7)."""
                psg = psacc.tile([D, PP], F32, tag="acc")
                for h in range(2):
                    cols = slice(512 * h, 512 * (h + 1))
                    nc.tensor.matmul(
                        psg[:, cols], sb_w[:, t, :], sb_peTo_b[:, cols],
                        start=True, stop=True,
                    )
                sigT = stagep.tile([D, PP], F32, tag="sigT", bufs=2)
                nc.scalar.activation(
                    sigT[:], psg[:], SIG, bias=sb_bT[:, t : t + 1]
                )
                nc.vector.tensor_mul(sb_gateT[t], sb_peTo_s[:], sigT[:])

            def stream_pairs(lhs8, matT, n_pairs, n_out, psum_tiles, eng,
                             ck_pairs, tag):
                """psum[d, :] += sum over k-pairs lhs8_pair.T @ matT chunk.

                matT: DRAM [128, n_pairs, 2, n_out] fp8; streamed in chunks
                of ck_pairs pairs on queue `eng` (own ring of 2 bufs).
                psum_tiles: list of [128, 512] psum col-chunks over n_out.
                """
                n512 = n_out // 512
                for c0 in range(0, n_pairs, ck_pairs):
                    cn = min(ck_pairs, n_pairs - c0)
                    chunk = rhsp.tile(
                        [128, ck_pairs, 2, n_out], FP8, tag=tag, name=tag
                    )
                    eng.dma_start(
                        chunk[:, :cn, :, :], matT[:, c0 : c0 + cn, :, :]
                    )
                    for kk in range(cn):
                        pair = c0 + kk
                        lhs_pair = lhs8[:, 2 * pair : 2 * pair + 2, :]
                        start = pair == 0
                        stop = pair == n_pairs - 1
                        for n in range(n512):
                            nc.tensor.matmul(
                                psum_tiles[n],
                                lhs_pair,
                                chunk[:, kk, :, 512 * n : 512 * (n + 1)],
                                start=start, stop=stop, perf_mode=DR,
                            )

            def to_nat_fp8(srcT, dst8, n_m):
                """PE-transpose [D, n_m*128] f32 srcT into natural fp8
                k-tiles dst8 [128, n_m, 128]."""
                for j in range(n_m // 4):
                    pst = pzp.tile([128, 4, D], F32, tag="pz")
                    for m in range(4):
                        col = (4 * j + m) * 128
                        nc.tensor.transpose(
                            pst[:, m, :], srcT[:, col : col + 128], sb_ident[:]
                        )
                    nc.scalar.copy(dst8[:, 4 * j : 4 * j + 4, :], pst[:])

            def allgather(cc_in, cc_out):
                nc.gpsimd.collective_compute(
                    "AllGather", BYPASS, replica_groups=RG,
                    ins=[cc_in[:].opt()], outs=[cc_out[:].opt()],
                )

            def load_full(cc_out, n_blk, name):
                """Gather rank blocks [128, n_blk, D] into [128, 8*n_blk, D]."""
                full = bigp.tile([128, NCORES * n_blk, D], FP8, tag="yfull",
                                 name=name)
                for r in range(NCORES):
                    nc.gpsimd.dma_start(
                        full[:, r * n_blk : (r + 1) * n_blk, :],
                        cc_out[r * 128 : (r + 1) * 128, :, :],
                    )
                return full

            # ---- phase A0: x gate ---------------------------------------
            gate_full(0)
            gate_own(0)

            # ---- phase B1: y_up = HG_up @ x (U-row shard) ---------------
            ps_yu = psacc.tile([D, UU], F32, tag="acc")
            stream_pairs(sb_gate8[0], UpT, KP // 2, UU, [ps_yu[:, 0:512]],
                         nc.sync, 4, "ck_up")
            yuT = stagep.tile([D, UU], F32, tag="ysb")
            nc.scalar.copy(yuT[:], ps_yu[:])
            yu8 = stagep.tile([128, UU // 128, D], FP8, tag="y8")
            to_nat_fp8(yuT, yu8, UU // 128)
            nc.gpsimd.dma_start(cc_yu_in[:], yu8[:])
            allgather(cc_yu_in, cc_yu_out)

            # ---- phase A1: s gate (overlaps AG1) ------------------------
            gate_full(1)
            gate_own(1)

            # ---- phase B2: y_tar = HG_poi_tar @ s (E-row shard) ---------
            ps_yt = psacc.tile([D, EE], F32, tag="acc")
            stream_pairs(sb_gate8[1], TarT, KP // 2, EE, [ps_yt[:, 0:512]],
                         nc.scalar, 4, "ck_tar")
            ytT = stagep.tile([D, EE], F32, tag="ysb")
            nc.scalar.copy(ytT[:], ps_yt[:])
            yt8 = stagep.tile([128, EE // 128, D], FP8, tag="y8")
            to_nat_fp8(ytT, yt8, EE // 128)
            nc.gpsimd.dma_start(cc_yt_in[:], yt8[:])
            allgather(cc_yt_in, cc_yt_out)

            # ---- phase A2: g gate ---------------------------------------
            gate_full(2)
            gate_own(2)

            # ---- phase B3: geo_pois = g + 0.4 * Geo @ g (P-row shard) ---
            ps_geo = psacc.tile([D, PP], F32, tag="acc")
            stream_pairs(sb_gate8[2], GeoT, KP // 2, PP,
                         [ps_geo[:, 0:512], ps_geo[:, 512:1024]],
                         nc.gpsimd, 2, "ck_geo")
            geoT_s = outp.tile([D, PP], F32, tag="out", name="geoT_s")
            nc.vector.scalar_tensor_tensor(
                geoT_s[:], ps_geo[:], S_GEO_OUT, sb_gateT[2][:], MULT, ADD
            )
            nc.sync.dma_start(poisT_o[1], geoT_s[:])
            geo8 = constp.tile([128, PP // 128, D], FP8, name="geo8")
            to_nat_fp8(geoT_s, geo8, PP // 128)

            # ---- gathered y_up ------------------------------------------
            yup_full = load_full(cc_yu_out, UU // 128, "yup_full")

            # ---- phase C1: hg_pois = x + HG_pu @ y_up (P-row shard) -----
            ps_hg = psacc.tile([D, PP], F32, tag="acc")
            stream_pairs(yup_full, PuT, KU // 2, PP,
                         [ps_hg[:, 0:512], ps_hg[:, 512:1024]],
                         nc.sync, 2, "ck_pu")
            hgT_s = outp.tile([D, PP], F32, tag="out", name="hgT_s")
            nc.vector.scalar_tensor_tensor(
                hgT_s[:], ps_hg[:], S_HG_OUT, sb_gateT[0][:], MULT, ADD
            )
            nc.sync.dma_start(poisT_o[0], hgT_s[:])
            hg8 = constp.tile([128, PP // 128, D], FP8, name="hg8")
            to_nat_fp8(hgT_s, hg8, PP // 128)

            # ---- gathered y_tar -----------------------------------------
            ytar_full = load_full(cc_yt_out, EE // 128, "ytar_full")

            # ---- phase C2: trans_pois = s + Src @ y_tar (P-row shard) ---
            ps_tr = psacc.tile([D, PP], F32, tag="acc")
            stream_pairs(ytar_full, SrcT, KU // 2, PP,
                         [ps_tr[:, 0:512], ps_tr[:, 512:1024]],
                         nc.scalar, 2, "ck_src")
            trT_s = outp.tile([D, PP], F32, tag="out", name="trT_s")
            nc.vector.scalar_tensor_tensor(
                trT_s[:], ps_tr[:], S_HG_OUT, sb_gateT[1][:], MULT, ADD
            )
            nc.scalar.dma_start(poisT_o[2], trT_s[:])

            # ---- phase D: user partials (P-col shard, host reduces) -----
            n_pairs_u = PP // 256
            for uc in range(U // 512):
                chunk = rhsp.tile([128, n_pairs_u, 2, 512], FP8, tag="urhs")
                nc.sync.dma_start(chunk[:], UpC[:, uc, :, :, :])
                ps_u = [
                    psacc.tile([D, 512], F32, tag="uacc", bufs=2, name="ps_u") for _ in range(2)
                ]
                for c0 in range(n_pairs_u):
                    for j, lhs8 in enumerate((hg8, geo8)):
                        nc.tensor.matmul(
                            ps_u[j][:], lhs8[:, 2 * c0 : 2 * c0 + 2, :],
                            chunk[:, c0, :, :],
                            start=(c0 == 0), stop=(c0 == n_pairs_u - 1),
                            perf_mode=DR,
                        )
                users_uc = outp.tile([D, 2, 512], BF16, tag="uout",
                                     name="users_uc")
                for j in range(2):
                    nc.vector.tensor_copy(users_uc[:, j, :], ps_u[j][:])
                nc.sync.dma_start(
                    usersT_o[:, :, 512 * uc : 512 * (uc + 1)], users_uc[:]
                )

    nc.compile()
    return nc


def _get_nc():
    if "nc" not in _CACHE:
        _CACHE["nc"] = _build_nc()
    return _CACHE["nc"]


def _pair_layout(matT, n_out):
    """[n_k*128, n_out] f32 -> fp8 paired k-tile layout [128, n_k/2, 2, n_out]."""
    n_k = matT.shape[0] // 128
    fp8 = ml_dtypes.float8_e4m3
    return np.ascontiguousarray(
        (matT * SM)
        .reshape(n_k // 2, 2, 128, n_out)
        .transpose(2, 0, 1, 3)
    ).astype(fp8)


def _shard_inputs(inputs):
    f32 = np.float32
    bf16 = ml_dtypes.bfloat16
    pe = np.asarray(inputs["poi_emb_weight"], f32)[:P]
    peT = np.ascontiguousarray(pe.T)
    peN_s = np.ascontiguousarray(
        (pe * SX).reshape(KP, 128, D).transpose(1, 0, 2)
    )
    wN = np.ascontiguousarray(
        np.stack(
            [
                np.asarray(inputs["w_gate_col"], f32),
                np.asarray(inputs["w_gate_seq"], f32),
                np.asarray(inputs["w_gate_geo"], f32),
            ]
        ).transpose(1, 0, 2)
    ).astype(bf16)
    b3 = np.stack(
        [
            np.asarray(inputs["b_gate_col"], f32)[0],
            np.asarray(inputs["b_gate_seq"], f32)[0],
            np.asarray(inputs["b_gate_geo"], f32)[0],
        ]
    )  # [3, D]
    # bias broadcast tile [128, 3, 4, D] (same b_t row in every partition
    # and every 128-row sub-tile of a 512-row gate chunk)
    bB = np.ascontiguousarray(
        np.broadcast_to(b3[None, :, None, :], (128, 3, 4, D))
    ).astype(bf16)
    bT3 = np.ascontiguousarray(b3.T)                           # [D, 3] f32
    ident = np.eye(D, dtype=f32)

    Up = np.asarray(inputs["HG_up"], f32)
    Pu = np.asarray(inputs["HG_pu"], f32)
    Tar = np.asarray(inputs["HG_poi_tar"], f32)
    Src = np.asarray(inputs["HG_poi_src"], f32)
    Geo = np.asarray(inputs["poi_geo_graph"], f32)

    in_maps = []
    for i in range(NCORES):
        rp = slice(PP * i, PP * (i + 1))
        ru = slice(UU * i, UU * (i + 1))
        re_ = slice(EE * i, EE * (i + 1))
        in_maps.append(
            {
                "peT": peT.astype(bf16),
                "peN_s": peN_s,
                "peTo_b": np.ascontiguousarray(peT[:, rp]).astype(bf16),
                "peTo_s": np.ascontiguousarray(peT[:, rp] * SX),
                "wN": wN,
                "bB": bB,
                "bT3": bT3,
                "ident": ident,
                "UpT": _pair_layout(Up[ru].T, UU),
                "TarT": _pair_layout(Tar[re_].T, EE),
                "PuT": _pair_layout(Pu[rp].T, PP),
                "SrcT": _pair_layout(Src[rp].T, PP),
                "GeoT": _pair_layout(Geo[rp].T, PP),
                "UpC": np.ascontiguousarray(
                    (Up[:, rp].T * SM)
                    .reshape(PP // 256, 2, 128, U // 512, 512)
                    .transpose(2, 3, 0, 1, 4)
                ).astype(ml_dtypes.float8_e4m3),
            }
        )
    return in_maps


def _assemble(results, user_idx):
    f32 = np.float32
    hg = np.empty((P, D), f32)
    geo = np.empty((P, D), f32)
    tr = np.empty((P, D), f32)
    users_acc = np.zeros((D, 2, U), f32)
    inv_sx = 1.0 / SX
    for i in range(NCORES):
        rp = slice(PP * i, PP * (i + 1))
        pois = results[i]["poisT_o"]
        hg[rp] = pois[0].T * inv_sx
        geo[rp] = pois[1].T * inv_sx
        tr[rp] = pois[2].T * inv_sx
        users_acc += results[i]["usersT_o"].astype(f32)
    users_acc *= S_USERS
    hgu = users_acc[:, 0, :].T
    geou = users_acc[:, 1, :].T
    idx = np.asarray(user_idx)
    return np.concatenate([hg, geo, tr, hgu[idx], geou[idx]], axis=0)


def _run(inputs, trace=False, **spmd_kwargs):
    nc = _get_nc()
    in_maps = _shard_inputs(inputs)
    res = run_bass_kernel_spmd(
        nc, in_maps, list(range(NCORES)), trace=trace, **spmd_kwargs
    )
    return _assemble(res.results, inputs["user_idx"]), res


def kernel(**inputs):
    return _run(inputs)[0]


if __name__ == "__main__":
    import pickle

    with open("/tmp/inputs.pkl", "rb") as f:
        inputs = pickle.load(f)
    out = kernel(**inputs)
    exp = np.load("/tmp/expected.npy")
    rel = np.linalg.norm(out - exp) / np.linalg.norm(exp)
    print("Relative error:", rel)
